# revision 1
# baseline (speedup 1.0000x reference)
"""Kandinsky5Attention Bass/Tile kernel for 8 Trainium2 NeuronCores.

Sharding: core = (batch b, head-group g): 2 batches x 4 groups of 4 heads.
Each core computes q/k/v for its 512 features of its batch, attention for
its 4 heads, and a partial output projection over its 512 contraction dims.
Host sums the 4 partials per batch and adds the output bias.

All on-device matmuls run in float32r (full PE rate, ~1e-4 rel err).
Weight columns are permuted per head (even dims then odd dims) so RoPE
operates on contiguous partition blocks [0:64] / [64:128].
"""
import math

import numpy as np

import concourse.bass as bass
import concourse.mybir as mybir
import concourse.tile as tile
from concourse import bacc
from concourse.bass_utils import run_bass_kernel_spmd

B, S, C, HD = 2, 2048, 2048, 128
H = C // HD            # 16 heads
HG = 4                 # head groups (cores per batch)
HPG = H // HG          # 4 heads per group
GF = HPG * HD          # 512 features per group
EPS = float(np.finfo(np.float32).eps)
N_CORES = 8
NCC = C // 128         # 16 contraction chunks
S_T1 = 256             # phase-1 s tile
N_ST1 = S // S_T1      # 8
SQ_T = 512             # attention q tile
N_SQ = S // SQ_T       # 4
N_SK = S // 128        # 16

F32 = mybir.dt.float32
F32R = mybir.dt.float32r
AF = mybir.ActivationFunctionType
ISCALE = 1.0 / math.sqrt(HD)


def build_program():
    nc = bacc.Bacc("TRN2", target_bir_lowering=False, debug=False,
                   num_devices=N_CORES)

    xt = nc.dram_tensor("xt", [C, S], F32, kind="ExternalInput")
    wqt = nc.dram_tensor("wqt", [C, GF], F32, kind="ExternalInput")
    wkt = nc.dram_tensor("wkt", [C, GF], F32, kind="ExternalInput")
    wvt = nc.dram_tensor("wvt", [C, GF], F32, kind="ExternalInput")
    wot = nc.dram_tensor("wot", [GF, C], F32, kind="ExternalInput")
    bqd = nc.dram_tensor("bq", [HPG, HD], F32, kind="ExternalInput")
    bkd = nc.dram_tensor("bk", [HPG, HD], F32, kind="ExternalInput")
    bvd = nc.dram_tensor("bv", [1, GF], F32, kind="ExternalInput")
    qnwd = nc.dram_tensor("qnw", [1, HD], F32, kind="ExternalInput")
    knwd = nc.dram_tensor("knw", [1, HD], F32, kind="ExternalInput")
    roped = nc.dram_tensor("rope", [2, 2, HD // 2, S], F32,
                           kind="ExternalInput")
    onesd = nc.dram_tensor("ones", [2, 128], F32, kind="ExternalInput")
    outd = nc.dram_tensor("out", [S, C], F32, kind="ExternalOutput")

    # DRAM scratch (qkv spill between phases)
    qtn = nc.dram_tensor("qtn_scr", [HPG, HD, S], F32R)
    ktn = nc.dram_tensor("ktn_scr", [HPG, HD, S], F32R)
    vsc = nc.dram_tensor("v_scr", [S, GF], F32R)

    xt_r = xt[:, :].rearrange("(cc p) s -> p cc s", p=128)
    wq_r = wqt[:, :].rearrange("(cc p) g -> p cc g", p=128)
    wk_r = wkt[:, :].rearrange("(cc p) g -> p cc g", p=128)
    wv_r = wvt[:, :].rearrange("(cc p) g -> p cc g", p=128)
    wo_r = wot[:, :].rearrange("(h p) c -> p h c", p=128)
    vs_r = vsc[:, :].rearrange("(t p) g -> p t g", p=128)

    with tile.TileContext(nc) as tc, \
            nc.allow_low_precision(reason="float32r is 4-byte; PE truncation only"):
        with tc.tile_pool(name="glob", bufs=1) as glob:
            ones_row = glob.tile([1, 128], F32R)       # lhsT for broadcasts
            nc.sync.dma_start(out=ones_row, in_=onesd[0:1, :].bitcast(F32R))
            ones_col = glob.tile([128, 1], F32R)       # lhsT for part-sums
            nc.sync.dma_start(out=ones_col,
                              in_=onesd[0:1, :].rearrange("o d -> d o").bitcast(F32R))
            qnw_t = glob.tile([1, HD], F32R)
            nc.sync.dma_start(out=qnw_t, in_=qnwd[:, :].bitcast(F32R))
            knw_t = glob.tile([1, HD], F32R)
            nc.sync.dma_start(out=knw_t, in_=knwd[:, :].bitcast(F32R))
            bv_t = glob.tile([1, GF], F32R)
            nc.sync.dma_start(out=bv_t, in_=bvd[:, :].bitcast(F32R))
            eps_t = glob.tile([1, 1], F32)
            nc.sync.dma_start(out=eps_t, in_=onesd[1:2, 0:1])
            bq_t = glob.tile([128, HPG], F32)
            nc.sync.dma_start(out=bq_t, in_=bqd[:, :].rearrange("h d -> d h"))
            bk_t = glob.tile([128, HPG], F32)
            nc.sync.dma_start(out=bk_t, in_=bkd[:, :].rearrange("h d -> d h"))

            # ---------------- Phase 1: QKV + RMSNorm + RoPE ----------------
            with (
                tc.tile_pool(name="p1w", bufs=1) as p1w,
                tc.tile_pool(name="p1x", bufs=2) as p1x,
                tc.tile_pool(name="p1t", bufs=3) as p1t,
                tc.tile_pool(name="p1ps", bufs=2, space="PSUM") as p1ps,
            ):
                wq_s = p1w.tile([128, NCC, GF], F32R)
                nc.sync.dma_start(out=wq_s, in_=wq_r.bitcast(F32R))
                wk_s = p1w.tile([128, NCC, GF], F32R)
                nc.sync.dma_start(out=wk_s, in_=wk_r.bitcast(F32R))
                wv_s = p1w.tile([128, NCC, GF], F32R)
                nc.sync.dma_start(out=wv_s, in_=wv_r.bitcast(F32R))

                for st in range(N_ST1):
                    sl = slice(st * S_T1, (st + 1) * S_T1)
                    xs = p1x.tile([128, NCC, S_T1], F32R, tag="xs")
                    nc.sync.dma_start(out=xs, in_=xt_r[:, :, sl].bitcast(F32R))
                    Ra = p1t.tile([128, S_T1], F32, bufs=2, tag="Ra",
                                  name=f"Ra_{st}")
                    nc.sync.dma_start(out=Ra[0:64, :], in_=roped[0, 0][:, sl])
                    nc.sync.dma_start(out=Ra[64:128, :], in_=roped[0, 1][:, sl])
                    Rb = p1t.tile([128, S_T1], F32, bufs=2, tag="Rb",
                                  name=f"Rb_{st}")
                    nc.sync.dma_start(out=Rb[0:64, :], in_=roped[1, 0][:, sl])
                    nc.sync.dma_start(out=Rb[64:128, :], in_=roped[1, 1][:, sl])

                    for w_s, nw_t, b_t, dst in (
                        (wq_s, qnw_t, bq_t, qtn),
                        (wk_s, knw_t, bk_t, ktn),
                    ):
                        for h in range(HPG):
                            hsl = slice(h * HD, (h + 1) * HD)
                            ps = p1ps.tile([128, S_T1], F32, tag="qk")
                            for cc in range(NCC):
                                nc.tensor.matmul(
                                    ps[:], w_s[:, cc, hsl], xs[:, cc, :],
                                    start=(cc == 0), stop=(cc == NCC - 1))
                            raw = p1t.tile([128, S_T1], F32, tag="raw")
                            nc.scalar.activation(raw[:], ps[:], AF.Identity,
                                                 bias=b_t[:, h:h + 1])
                            sq2 = p1t.tile([128, S_T1], F32R, tag="sq2")
                            nc.vector.tensor_mul(sq2[:], raw[:], raw[:])
                            ssq = p1ps.tile([1, S_T1], F32, tag="ssq")
                            nc.tensor.matmul(ssq[:], ones_col[:], sq2[:])
                            sd = p1t.tile([1, S_T1], F32, tag="sd")
                            nc.scalar.activation(sd[:], ssq[:], AF.Sqrt,
                                                 scale=1.0 / HD,
                                                 bias=eps_t[:])
                            rs = p1t.tile([1, S_T1], F32R, tag="rs")
                            nc.vector.reciprocal(rs[:], sd[:])
                            sc = p1ps.tile([128, S_T1], F32, tag="sc", bufs=1)
                            nc.tensor.matmul(sc[:], nw_t[:], rs[:])
                            qn = p1t.tile([128, S_T1], F32, tag="qn")
                            nc.vector.tensor_mul(qn[:], raw[:], sc[:])
                            # rope: ta=[R00*qe; R01*qo], tb=[R10*qe; R11*qo]
                            qr = p1t.tile([128, S_T1], F32R, tag="qr")
                            ta = p1t.tile([128, S_T1], F32, tag="rta")
                            tb = p1t.tile([128, S_T1], F32, tag="rtb")
                            nc.vector.tensor_mul(ta[:], Ra[:], qn[:])
                            nc.vector.tensor_mul(tb[:], Rb[:], qn[:])
                            # partition swap via DMA: m1 = [ta_hi ; tb_lo]
                            m1 = p1t.tile([128, S_T1], F32, tag="rm1")
                            nc.sync.dma_start(out=m1[0:64, :], in_=ta[64:128, :])
                            nc.sync.dma_start(out=m1[64:128, :], in_=tb[0:64, :])
                            nc.vector.tensor_add(qr[0:64, :], ta[0:64, :],
                                                 m1[0:64, :])
                            nc.vector.tensor_add(qr[64:128, :], tb[64:128, :],
                                                 m1[64:128, :])
                            nc.sync.dma_start(out=dst[h, :, sl], in_=qr[:])

                    for ss in range(S_T1 // 128):
                        vp = p1ps.tile([128, GF], F32, tag="v")
                        ssl = slice(ss * 128, (ss + 1) * 128)
                        for cc in range(NCC):
                            nc.tensor.matmul(vp[:], xs[:, cc, ssl], wv_s[:, cc, :],
                                             start=(cc == 0), stop=False)
                        nc.tensor.matmul(vp[:], ones_row[:], bv_t[:],
                                         start=False, stop=True)
                        ve = p1t.tile([128, GF], F32R, tag="ve")
                        nc.scalar.activation(ve[:], vp[:], AF.Copy)
                        r0 = st * S_T1 + ss * 128
                        nc.sync.dma_start(out=vsc[r0:r0 + 128, :], in_=ve[:])

            # ---------------- Phase 2: attention ----------------
            with tc.tile_pool(name="p23", bufs=1) as p23:
              oT = p23.tile([128, HPG, S], F32R)
              with (
                tc.tile_pool(name="p2h", bufs=2) as p2h,
                tc.tile_pool(name="p2e", bufs=1) as p2e,
                tc.tile_pool(name="p2t", bufs=2) as p2t,
                tc.tile_pool(name="p2ps", bufs=2, space="PSUM") as p2ps,
              ):
                for h in range(HPG):
                    kh = p2h.tile([128, S], F32R, tag="kh")
                    nc.sync.dma_start(out=kh, in_=ktn[h])
                    qh = p2h.tile([128, S], F32R, tag="qh")
                    nc.sync.dma_start(out=qh, in_=qtn[h])
                    vh = p2h.tile([128, N_SK, HD], F32R, tag="vh")
                    nc.sync.dma_start(
                        out=vh, in_=vs_r[:, :, h * HD:(h + 1) * HD])
                    for sq in range(N_SQ):
                        qsl = slice(sq * SQ_T, (sq + 1) * SQ_T)
                        es = [p2e.tile([128, SQ_T], F32R, tag="es", bufs=32,
                                       name=f"es{h}_{sq}_{i}")
                              for i in range(N_SK)]
                        z_ps = p2ps.tile([1, SQ_T], F32, tag="z")
                        for sk in range(N_SK):
                            sc_ps = p2ps.tile([128, SQ_T], F32, tag="scs")
                            nc.tensor.matmul(
                                sc_ps[:], kh[:, sk * 128:(sk + 1) * 128],
                                qh[:, qsl])
                            nc.scalar.activation(es[sk][:], sc_ps[:], AF.Exp,
                                                 scale=ISCALE)
                            nc.tensor.matmul(z_ps[:], ones_col[:], es[sk][:],
                                             start=(sk == 0),
                                             stop=(sk == N_SK - 1))
                        rz = p2t.tile([1, SQ_T], F32R, tag="rz")
                        nc.vector.reciprocal(rz[:], z_ps[:])
                        o_ps = p2ps.tile([128, SQ_T], F32, tag="o")
                        for sk in range(N_SK):
                            nc.tensor.matmul(o_ps[:], vh[:, sk, :], es[sk][:],
                                             start=(sk == 0),
                                             stop=(sk == N_SK - 1))
                        rzb = p2ps.tile([128, SQ_T], F32, tag="rzb", bufs=1)
                        nc.tensor.matmul(rzb[:], ones_row[:], rz[:])
                        oe = p2t.tile([128, SQ_T], F32, tag="oe")
                        nc.scalar.activation(oe[:], o_ps[:], AF.Copy)
                        nc.vector.tensor_mul(oT[:, h, qsl], oe[:], rzb[:])

              # ---------------- Phase 3: output projection ----------------
              with (
                tc.tile_pool(name="p3w", bufs=1) as p3w,
                tc.tile_pool(name="p3t", bufs=3) as p3t,
                tc.tile_pool(name="p3ps", bufs=2, space="PSUM") as p3ps,
              ):
                wo_s = p3w.tile([128, HPG, C], F32R)
                nc.sync.dma_start(out=wo_s, in_=wo_r.bitcast(F32R))
                for st in range(S // 128):
                    stsl = slice(st * 128, (st + 1) * 128)
                    for j in range(C // 512):
                        jsl = slice(j * 512, (j + 1) * 512)
                        op = p3ps.tile([128, 512], F32, tag="op")
                        for h in range(HPG):
                            nc.tensor.matmul(op[:], oT[:, h, stsl],
                                             wo_s[:, h, jsl],
                                             start=(h == 0), stop=(h == HPG - 1))
                        oe = p3t.tile([128, 512], F32, tag="oe3")
                        nc.scalar.activation(oe[:], op[:], AF.Copy)
                        nc.sync.dma_start(out=outd[stsl, jsl], in_=oe[:])

    nc.compile()
    return nc


_PROGRAM = None


def _get_program():
    global _PROGRAM
    if _PROGRAM is None:
        _PROGRAM = build_program()
    return _PROGRAM


def _perm128():
    # even head dims then odd head dims
    return np.concatenate([np.arange(0, HD, 2), np.arange(1, HD, 2)])


def prepare_in_maps(hidden_states, rotary_emb, wq, bq, wk, bk, wv, bv,
                    q_norm_w, k_norm_w, wo, bo):
    f32 = np.float32
    hidden_states = np.asarray(hidden_states, f32)
    rotary_emb = np.asarray(rotary_emb, f32)
    wq, bq = np.asarray(wq, f32), np.asarray(bq, f32)
    wk, bk = np.asarray(wk, f32), np.asarray(bk, f32)
    wv, bv = np.asarray(wv, f32), np.asarray(bv, f32)
    wo = np.asarray(wo, f32)
    q_norm_w, k_norm_w = np.asarray(q_norm_w, f32), np.asarray(k_norm_w, f32)

    p128 = _perm128()
    # rope tensors R[r][c][i, s]
    rope = np.ascontiguousarray(
        rotary_emb[0, :, 0, :, :, :].transpose(2, 3, 1, 0))  # [2,2,64,S]
    ones = np.ones((2, 128), f32)
    ones[1, :] = EPS
    qnw = np.ascontiguousarray(q_norm_w[p128])[None, :]
    knw = np.ascontiguousarray(k_norm_w[p128])[None, :]

    wqT = wq.T  # [in C, out C]
    wkT = wk.T
    wvT = wv.T
    woT = wo.T  # [d, j]

    in_maps = []
    for core in range(N_CORES):
        b, g = divmod(core, HG)
        base = g * GF
        cols = np.concatenate(
            [base + hh * HD + p128 for hh in range(HPG)])
        xt = np.ascontiguousarray(hidden_states[b].T)
        in_maps.append({
            "xt": xt,
            "wqt": np.ascontiguousarray(wqT[:, cols]),
            "wkt": np.ascontiguousarray(wkT[:, cols]),
            "wvt": np.ascontiguousarray(wvT[:, base:base + GF]),
            "wot": np.ascontiguousarray(woT[base:base + GF, :]),
            "bq": np.ascontiguousarray(bq[cols]).reshape(HPG, HD),
            "bk": np.ascontiguousarray(bk[cols]).reshape(HPG, HD),
            "bv": np.ascontiguousarray(bv[base:base + GF])[None, :],
            "qnw": qnw,
            "knw": knw,
            "rope": rope,
            "ones": ones,
        })
    return in_maps


def combine_results(results, bo):
    bo = np.asarray(bo, np.float32)
    out = np.zeros((B, S, C), np.float32)
    for core in range(N_CORES):
        b = core // HG
        out[b] += results[core]["out"]
    out += bo
    return out


def kernel(hidden_states, rotary_emb, wq, bq, wk, bk, wv, bv,
           q_norm_w, k_norm_w, wo, bo):
    nc = _get_program()
    in_maps = prepare_in_maps(hidden_states, rotary_emb, wq, bq, wk, bk,
                              wv, bv, q_norm_w, k_norm_w, wo, bo)
    res = run_bass_kernel_spmd(nc, in_maps, list(range(N_CORES)))
    return combine_results(res.results, bo)



# revision 13
# speedup vs baseline: 14880.0451x; 14880.0451x over previous
"""Kandinsky5Attention Bass/Tile kernel for 8 Trainium2 NeuronCores.

Sharding: core = (batch b, head-group g): 2 batches x 4 groups of 4 heads.
Each core computes q/k/v for its 512 features of its batch, attention for
its 4 heads, and a partial output projection over its 512 contraction dims.
Host sums the 4 partials per batch and adds the output bias.

v2 design:
 - all matmuls bf16 (host-cast inputs), fp32 PSUM accumulation
 - q/k/v and attention outputs stay SBUF-resident (no DRAM spill)
 - RMSNorm weight and 1/sqrt(HD) folded into rope coefficients on the host
 - per-position q norm scale applied via gpsimd partition_broadcast + mul;
   k norm scale folded into the exp's per-partition scale vector
 - 1/z via reciprocal_approx_fast; output projection interleaved per q-block
"""
import math

import numpy as np
import ml_dtypes

import concourse.bass as bass
import concourse.mybir as mybir
import concourse.tile as tile
from concourse import bacc
from concourse.bass_utils import run_bass_kernel_spmd

B, S, C, HD = 2, 2048, 2048, 128
H = C // HD            # 16 heads
HG = 4                 # head groups (cores per batch)
HPG = H // HG          # 4 heads per group
GF = HPG * HD          # 512 features per group
EPS = float(np.finfo(np.float32).eps)
N_CORES = 8
NCC = C // 128         # 16 contraction chunks
ST = 512               # phase-1 s tile
N_ST = S // ST         # 4
SQ_T = 1024            # phase-2 query block
N_SQ = S // SQ_T       # 2
N_SK = S // 128        # 16 key chunks

F32 = mybir.dt.float32
BF16 = mybir.dt.bfloat16
AF = mybir.ActivationFunctionType
ALU = mybir.AluOpType
ISCALE = 1.0 / math.sqrt(HD)


def build_program():
    nc = bacc.Bacc("TRN2", target_bir_lowering=False, debug=False,
                   num_devices=N_CORES)

    xt = nc.dram_tensor("xt", [C, S], BF16, kind="ExternalInput")
    wqt = nc.dram_tensor("wqt", [C, GF], BF16, kind="ExternalInput")
    wkt = nc.dram_tensor("wkt", [C, GF], BF16, kind="ExternalInput")
    wvt = nc.dram_tensor("wvt", [C, GF], BF16, kind="ExternalInput")
    wot = nc.dram_tensor("wot", [GF, C], BF16, kind="ExternalInput")
    bqd = nc.dram_tensor("bq", [HPG, HD], F32, kind="ExternalInput")
    bkd = nc.dram_tensor("bk", [HPG, HD], F32, kind="ExternalInput")
    bvd = nc.dram_tensor("bv", [1, GF], BF16, kind="ExternalInput")
    ropeqd = nc.dram_tensor("ropeq", [2, 128, S], BF16, kind="ExternalInput")
    ropekd = nc.dram_tensor("ropek", [2, 128, S], BF16, kind="ExternalInput")
    outd = nc.dram_tensor("out", [S, C], F32, kind="ExternalOutput")
    rkscr = nc.dram_tensor("rk_scr", [HPG, S], F32)  # transpose bounce

    xt_r = xt[:, :].rearrange("(cc p) s -> p cc s", p=128)
    wq_r = wqt[:, :].rearrange("(cc p) g -> p cc g", p=128)
    wk_r = wkt[:, :].rearrange("(cc p) g -> p cc g", p=128)
    wv_r = wvt[:, :].rearrange("(cc p) g -> p cc g", p=128)
    wo_r = wot[:, :].rearrange("(h p) c -> p h c", p=128)

    with tile.TileContext(nc) as tc, \
            nc.allow_low_precision(reason="bf16 matmuls within rel-err budget"):
        with tc.tile_pool(name="glob", bufs=1) as glob:
            ones_col = glob.tile([128, 1], BF16)
            nc.vector.memset(ones_col[:], 1.0)
            ones_row = glob.tile([1, 128], BF16)
            nc.vector.memset(ones_row[:], 1.0)
            eps_t = glob.tile([1, 1], F32)
            nc.vector.memset(eps_t[:], EPS)
            bq_t = glob.tile([128, HPG], F32)
            nc.sync.dma_start(out=bq_t, in_=bqd[:, :].rearrange("h d -> d h"))
            bk_t = glob.tile([128, HPG], F32)
            nc.sync.dma_start(out=bk_t, in_=bkd[:, :].rearrange("h d -> d h"))
            bv_t = glob.tile([1, GF], BF16)
            nc.sync.dma_start(out=bv_t, in_=bvd[:, :])
            ropeq_a = glob.tile([128, S], BF16)
            nc.sync.dma_start(out=ropeq_a, in_=ropeqd[0])
            ropeq_b = glob.tile([128, S], BF16)
            nc.sync.dma_start(out=ropeq_b, in_=ropeqd[1])
            ropek_a = glob.tile([128, S], BF16)
            nc.sync.dma_start(out=ropek_a, in_=ropekd[0])
            ropek_b = glob.tile([128, S], BF16)
            nc.sync.dma_start(out=ropek_b, in_=ropekd[1])

            q_sb = glob.tile([128, HPG, S], BF16)
            k_sb = glob.tile([128, HPG, S], BF16)
            v_sb = glob.tile([128, N_SK, GF], BF16)
            rq_rows = glob.tile([128, S], BF16)  # head h at partition 32*h
            rk_rows = glob.tile([128, S], F32)  # head h at partition 32*h
            rkT = glob.tile([128, HPG, N_SK], F32)
            wo_s = glob.tile([128, HPG, C], BF16)
            oT = [glob.tile([128, HPG, SQ_T], BF16, name=f"oT{i}")
                  for i in range(N_SQ)]

            # ---------------- Phase 1: QKV + RMSNorm + RoPE ----------------
            with (
                tc.tile_pool(name="p1w", bufs=1) as p1w,
                tc.tile_pool(name="p1x", bufs=2) as p1x,
                tc.tile_pool(name="p1t", bufs=2) as p1t,
                tc.tile_pool(name="p1ps", bufs=3, space="PSUM") as p1ps,
                tc.tile_pool(name="p1psv", bufs=2, space="PSUM") as p1psv,
                tc.tile_pool(name="p1pss", bufs=2, space="PSUM") as p1pss,
            ):
                wk_s = p1w.tile([128, NCC, GF], BF16)
                wq_s = p1w.tile([128, NCC, GF], BF16)
                wv_s = p1w.tile([128, NCC, GF], BF16)
                for cc in range(NCC):
                    nc.sync.dma_start(out=wk_s[:, cc, :], in_=wk_r[:, cc, :])
                    nc.sync.dma_start(out=wv_s[:, cc, :], in_=wv_r[:, cc, :])
                    nc.sync.dma_start(out=wq_s[:, cc, :], in_=wq_r[:, cc, :])
                # wo needed only in phase 3; issue after phase-1 weights
                for h in range(HPG):
                    nc.sync.dma_start(out=wo_s[:, h, :], in_=wo_r[:, h, :])

                for st in range(N_ST):
                    sl = slice(st * ST, (st + 1) * ST)
                    xs = p1x.tile([128, NCC, ST], BF16, tag="xs")
                    for cc in range(NCC):
                        nc.sync.dma_start(out=xs[:, cc, :], in_=xt_r[:, cc, sl])

                    for w_s, rows, ra, rb, b_t, is_q in (
                        (wk_s, rk_rows, ropek_a, ropek_b, bk_t, False),
                        (wq_s, rq_rows, ropeq_a, ropeq_b, bq_t, True),
                    ):
                        dst = q_sb if is_q else k_sb
                        for h in range(HPG):
                            hsl = slice(h * HD, (h + 1) * HD)
                            ps = p1ps.tile([128, ST], F32, tag="ps")
                            for cc in range(NCC):
                                nc.tensor.matmul(
                                    ps[:], w_s[:, cc, hsl], xs[:, cc, :],
                                    start=(cc == 0), stop=(cc == NCC - 1))
                            raw = p1t.tile([128, ST], BF16, tag="raw")
                            nc.scalar.activation(raw[:], ps[:], AF.Identity,
                                                 bias=b_t[:, h:h + 1])
                            sq2 = p1t.tile([128, ST], BF16, tag="sq2")
                            nc.vector.tensor_mul(sq2[:], raw[:], raw[:])
                            ssq = p1pss.tile([1, ST], F32, tag="ssq")
                            nc.tensor.matmul(ssq[:], ones_col[:], sq2[:])
                            # rsqrt(ms + eps) = exp(-0.5 * ln(ms + eps));
                            # ln/exp share one ACT table set with phase-2 exp
                            lms = p1t.tile([1, ST], F32, tag="lms")
                            nc.scalar.activation(lms[:], ssq[:], AF.Ln,
                                                 scale=1.0 / HD,
                                                 bias=eps_t[:])
                            nc.scalar.activation(rows[32 * h:32 * h + 1, sl],
                                                 lms[:], AF.Exp, scale=-0.5)
                            # rope: out_lo = ta_lo + ta_hi ; out_hi = tb_hi + tb_lo
                            ta = p1t.tile([128, ST], BF16, tag="ta")
                            tb = p1t.tile([128, ST], BF16, tag="tb")
                            nc.vector.tensor_mul(ta[:], ra[:, sl], raw[:])
                            nc.vector.tensor_mul(tb[:], rb[:, sl], raw[:])
                            m1 = p1t.tile([128, ST], BF16, tag="m1")
                            nc.sync.dma_start(out=m1[0:64, :], in_=ta[64:128, :])
                            nc.sync.dma_start(out=m1[64:128, :], in_=tb[0:64, :])
                            if is_q:
                                qt = p1t.tile([128, ST], BF16, tag="qt")
                                nc.vector.tensor_add(qt[0:64, :], ta[0:64, :],
                                                     m1[0:64, :])
                                nc.vector.tensor_add(qt[64:128, :],
                                                     tb[64:128, :],
                                                     m1[64:128, :])
                                rqb = p1t.tile([128, ST], BF16, tag="rqb")
                                nc.gpsimd.partition_broadcast(
                                    rqb[:], rows[32 * h:32 * h + 1, sl])
                                nc.vector.tensor_mul(q_sb[:, h, sl], qt[:],
                                                     rqb[:])
                            else:
                                nc.vector.tensor_add(k_sb[0:64, h, sl],
                                                     ta[0:64, :], m1[0:64, :])
                                nc.vector.tensor_add(k_sb[64:128, h, sl],
                                                     tb[64:128, :],
                                                     m1[64:128, :])

                    for j in range(ST // 128):
                        vp = p1psv.tile([128, GF], F32, tag="vp")
                        jsl = slice(j * 128, (j + 1) * 128)
                        for cc in range(NCC):
                            nc.tensor.matmul(vp[:], xs[:, cc, jsl],
                                             wv_s[:, cc, :],
                                             start=(cc == 0), stop=False)
                        nc.tensor.matmul(vp[:], ones_row[:], bv_t[:],
                                         start=False, stop=True)
                        nc.scalar.activation(
                            v_sb[:, st * (ST // 128) + j, :], vp[:], AF.Copy)

                # k norm scales, transposed to [key-partition, chunk] layout
                # via a DRAM bounce (SBUF APs can't partition-split free dims)
                for h in range(HPG):
                    nc.sync.dma_start(out=rkscr[h:h + 1, :],
                                      in_=rk_rows[32 * h:32 * h + 1, :])
                    nc.sync.dma_start(
                        out=rkT[:, h, :],
                        in_=rkscr[h:h + 1, :].rearrange(
                            "o (c p) -> p (c o)", p=128))

            # -------- Phase 2 + 3: attention + output projection --------
            with (
                tc.tile_pool(name="p2e", bufs=2) as p2e,
                tc.tile_pool(name="p2t", bufs=2) as p2t,
                tc.tile_pool(name="p3t", bufs=3) as p3t,
                tc.tile_pool(name="p2sc", bufs=3, space="PSUM") as p2sc,
                tc.tile_pool(name="p2z", bufs=1, space="PSUM") as p2z,
                tc.tile_pool(name="p2o", bufs=2, space="PSUM") as p2o,
                tc.tile_pool(name="p3ps", bufs=2, space="PSUM") as p3ps,
            ):
                NJQ = SQ_T // 512  # 512-wide query sub-blocks
                for sq in range(N_SQ):
                    q0 = sq * SQ_T
                    for h in range(HPG):
                        es = p2e.tile([128, N_SK, SQ_T], BF16, tag="es")
                        for sk in range(N_SK):
                            ksl = slice(sk * 128, (sk + 1) * 128)
                            for j in range(NJQ):
                                scj = p2sc.tile([128, 512], F32, tag="sc",
                                                name=f"sc{sq}_{h}_{sk}_{j}")
                                nc.tensor.matmul(
                                    scj[:], k_sb[:, h, ksl],
                                    q_sb[:, h, q0 + j * 512:q0 + (j + 1) * 512])
                                nc.scalar.activation(
                                    es[:, sk, j * 512:(j + 1) * 512], scj[:],
                                    AF.Exp, scale=rkT[:, h, sk:sk + 1])
                        # z rows live at partitions 0/32 of one PSUM bank
                        # (matmul out base partition must be 0, 32, or 64)
                        z_ps = p2z.tile([64, 512], F32, tag="z")
                        for j in range(NJQ):
                            for sk in range(N_SK):
                                nc.tensor.matmul(
                                    z_ps[32 * j:32 * j + 1, :], ones_col[:],
                                    es[:, sk, j * 512:(j + 1) * 512],
                                    start=(sk == 0), stop=(sk == N_SK - 1))
                        rz = p2t.tile([64, 512], F32, tag="rz")
                        rz16 = p2t.tile([64, 512], BF16, tag="rz16")
                        rzb = p2t.tile([128, NJQ, 512], BF16, tag="rzb")
                        for j in range(NJQ):
                            nc.vector.reciprocal_approx_fast(
                                out=rz[32 * j:32 * j + 1, :],
                                in_=z_ps[32 * j:32 * j + 1, :])
                            nc.vector.tensor_copy(rz16[32 * j:32 * j + 1, :],
                                                  rz[32 * j:32 * j + 1, :])
                            nc.gpsimd.partition_broadcast(
                                rzb[:, j, :], rz16[32 * j:32 * j + 1, :])
                        for j in range(NJQ):
                            o_ps = p2o.tile([128, 512], F32, tag="o")
                            jq = slice(j * 512, (j + 1) * 512)
                            for sk in range(N_SK):
                                nc.tensor.matmul(
                                    o_ps[:], v_sb[:, sk, h * HD:(h + 1) * HD],
                                    es[:, sk, jq],
                                    start=(sk == 0), stop=(sk == N_SK - 1))
                            nc.vector.scalar_tensor_tensor(
                                out=oT[sq][:, h, jq], in0=o_ps[:], scalar=1.0,
                                in1=rzb[:, j, :], op0=ALU.mult, op1=ALU.mult)

                    # phase 3 for this query block
                    for r in range(SQ_T // 128):
                        rsl = slice(q0 + r * 128, q0 + (r + 1) * 128)
                        orl = slice(r * 128, (r + 1) * 128)
                        for j in range(C // 512):
                            jsl = slice(j * 512, (j + 1) * 512)
                            op = p3ps.tile([128, 512], F32, tag="op")
                            for h in range(HPG):
                                nc.tensor.matmul(op[:], oT[sq][:, h, orl],
                                                 wo_s[:, h, jsl],
                                                 start=(h == 0),
                                                 stop=(h == HPG - 1))
                            oe3 = p3t.tile([128, 512], F32, tag="oe3")
                            nc.vector.tensor_copy(oe3[:], op[:])
                            nc.sync.dma_start(out=outd[rsl, jsl], in_=oe3[:])

    nc.compile()
    return nc


_PROGRAM = None


def _get_program():
    global _PROGRAM
    if _PROGRAM is None:
        _PROGRAM = build_program()
    return _PROGRAM


def _perm128():
    # even head dims then odd head dims
    return np.concatenate([np.arange(0, HD, 2), np.arange(1, HD, 2)])


def _rope_tiles(rotary_emb, norm_w, extra_scale):
    """Fold per-dim norm weight (and optional score scale) into rope coeffs.

    Returns [2, 128, S]: [a/b, partition, s] where partitions 0:64 multiply
    the even input dims and 64:128 the odd input dims.
    """
    f32 = np.float32
    Rt = np.asarray(rotary_emb, f32)[0, :, 0].transpose(1, 2, 3, 0)  # [64,2,2,S]
    w = np.asarray(norm_w, f32) * extra_scale
    we = w[0::2][:, None]
    wo = w[1::2][:, None]
    out = np.empty((2, 128, S), f32)
    out[0, 0:64] = Rt[:, 0, 0, :] * we
    out[0, 64:128] = Rt[:, 0, 1, :] * wo
    out[1, 0:64] = Rt[:, 1, 0, :] * we
    out[1, 64:128] = Rt[:, 1, 1, :] * wo
    return out.astype(ml_dtypes.bfloat16)


def prepare_in_maps(hidden_states, rotary_emb, wq, bq, wk, bk, wv, bv,
                    q_norm_w, k_norm_w, wo, bo):
    f32 = np.float32
    bf16 = ml_dtypes.bfloat16
    hidden_states = np.asarray(hidden_states, f32)
    wq, bq = np.asarray(wq, f32), np.asarray(bq, f32)
    wk, bk = np.asarray(wk, f32), np.asarray(bk, f32)
    wv, bv = np.asarray(wv, f32), np.asarray(bv, f32)
    wo = np.asarray(wo, f32)

    p128 = _perm128()
    ropeq = _rope_tiles(rotary_emb, q_norm_w, ISCALE)
    ropek = _rope_tiles(rotary_emb, k_norm_w, 1.0)

    wqT = wq.T  # [in C, out C]
    wkT = wk.T
    wvT = wv.T
    woT = wo.T  # [d, j]

    in_maps = []
    for core in range(N_CORES):
        b, g = divmod(core, HG)
        base = g * GF
        cols = np.concatenate(
            [base + hh * HD + p128 for hh in range(HPG)])
        xtb = np.ascontiguousarray(hidden_states[b].T).astype(bf16)
        in_maps.append({
            "xt": xtb,
            "wqt": np.ascontiguousarray(wqT[:, cols]).astype(bf16),
            "wkt": np.ascontiguousarray(wkT[:, cols]).astype(bf16),
            "wvt": np.ascontiguousarray(wvT[:, base:base + GF]).astype(bf16),
            "wot": np.ascontiguousarray(woT[base:base + GF, :]).astype(bf16),
            "bq": np.ascontiguousarray(bq[cols]).reshape(HPG, HD),
            "bk": np.ascontiguousarray(bk[cols]).reshape(HPG, HD),
            "bv": np.ascontiguousarray(bv[base:base + GF])[None, :].astype(bf16),
            "ropeq": ropeq,
            "ropek": ropek,
        })
    return in_maps


def combine_results(results, bo):
    bo = np.asarray(bo, np.float32)
    out = np.zeros((B, S, C), np.float32)
    for core in range(N_CORES):
        b = core // HG
        out[b] += results[core]["out"]
    out += bo
    return out


def kernel(hidden_states, rotary_emb, wq, bq, wk, bk, wv, bv,
           q_norm_w, k_norm_w, wo, bo):
    nc = _get_program()
    in_maps = prepare_in_maps(hidden_states, rotary_emb, wq, bq, wk, bk,
                              wv, bv, q_norm_w, k_norm_w, wo, bo)
    res = run_bass_kernel_spmd(nc, in_maps, list(range(N_CORES)))
    return combine_results(res.results, bo)
